# revision 30
# baseline (speedup 1.0000x reference)
"""DeepSeekMoE Trainium2 kernel (8 NeuronCores, expert-parallel).

Strategy
--------
Expert-parallel sharding (per the sharding hint): core c owns routed expert c
plus 1/8 of the tokens for the replicated shared expert.

The host performs only *integer* dispatch decisions (argmax top-2 of the
router logits) to decide token placement, gathers each expert's tokens
(padded to a fixed capacity CAP), and scatter-adds the per-expert outputs
back into the full output.  Every floating-point value that contributes to
the output is computed on device:

  - each core re-computes the router logits for its gathered tokens in full
    fp32 and derives the renormalized top-2 combine weight of *its own*
    expert via the sigmoid-of-logit-gap identity (p1/(p1+p2) =
    sigmoid(l1-l2)), selected with a one-hot expert column,
  - mm1: hT[j] = gelu(w1.T-block @ xT) with the expert's w1 resident in
    SBUF (bf16), accumulated over 8 k-tiles in PSUM,
  - mm2: y[t,:] = (hT.T @ w2) * w_comb[t], w2 SBUF-resident (bf16),
  - the shared expert runs the same pipeline on a contiguous 512-token
    slice with sw1/sw2 streamed (each tile used once).

Token layout per core: CAP=1152 gathered expert tokens (max real count is
1091 for the bench input; zero rows pad -- they produce exactly zero output)
processed as 512/512/128 chunks, plus 512 shared-slice tokens.  All GEMMs
run in bf16 (rel err ~3e-3, well inside the 2e-2 gate); the router runs in
fp32 because top-2 selection is precision critical.

Scheduling: input DMAs are spread across the three DMA paths (SP-HWDGE,
Act-HWDGE, Pool-SWDGE) with w1/xgb split per k-tile so the first mm1
matmul issues ~2us in; the 128-token tail of expert mm2 runs last so the
final drain is two small tiles; PSUM phases use 4-bank groups so adjacent
phases overlap inside the 8-bank budget.

Host combine: out[slice_c] = ys_c (shared), then out[idx_c] += yg_c for each
core -- index placement + the unavoidable unshard additions only.
"""

import sys

sys.path.insert(0, "/opt/trn_rl_repo")

from contextlib import ExitStack

import numpy as np
import ml_dtypes

import concourse.bass as bass  # noqa: F401  (engine types resolve through bacc)
import concourse.tile as tile
from concourse import bacc, mybir
from concourse.alu_op_type import AluOpType
from concourse.bass_utils import run_bass_kernel_spmd

F32 = mybir.dt.float32
BF16 = mybir.dt.bfloat16
BF = ml_dtypes.bfloat16
AF = mybir.ActivationFunctionType
X = mybir.AxisListType.X

D, H, E = 1024, 2048, 8
B, S = 2, 2048
T = B * S
NCORES = 8
SH = T // NCORES          # 512 shared-expert tokens per core
KD = D // 128             # 8 k-tiles over D
KH = H // 128             # 16 k-tiles over H
DEFAULT_CAP = 1152        # >= max per-expert token count (1091 for bench key)


def _chunks(cap):
    """Token chunks of <=512 (PSUM free-dim limit) covering [0, cap)."""
    out = []
    t0 = 0
    while t0 < cap:
        nt = min(512, cap - t0)
        out.append((t0, nt))
        t0 += nt
    return out


def build_program(cap: int, has_b1: bool, has_b2: bool, has_rb: bool):
    nc = bacc.Bacc("TRN2", debug=False)
    MTE = cap // 128

    xgt_f = nc.dram_tensor("xgt_f", [D, cap], F32, kind="ExternalInput").ap()
    xgt_b = nc.dram_tensor("xgt_b", [D, cap], BF16, kind="ExternalInput").ap()
    xst_b = nc.dram_tensor("xst_b", [D, SH], BF16, kind="ExternalInput").ap()
    # rw arrives host-pre-permuted to [128, KD*E] so its DMA is one
    # 256B-line transfer instead of 1024 32B descriptors.
    rw = nc.dram_tensor("rw", [128, KD * E], F32, kind="ExternalInput").ap()
    rb = nc.dram_tensor("rb", [1, E], F32, kind="ExternalInput").ap()
    w1 = nc.dram_tensor("w1", [D, H], BF16, kind="ExternalInput").ap()
    w2 = nc.dram_tensor("w2", [H, D], BF16, kind="ExternalInput").ap()
    sw1 = nc.dram_tensor("sw1", [D, H], BF16, kind="ExternalInput").ap()
    sw2 = nc.dram_tensor("sw2", [H, D], BF16, kind="ExternalInput").ap()
    b1 = nc.dram_tensor("b1", [1, H], BF16, kind="ExternalInput").ap()
    b2 = nc.dram_tensor("b2", [1, D], BF16, kind="ExternalInput").ap()
    sb1 = nc.dram_tensor("sb1", [1, H], BF16, kind="ExternalInput").ap()
    sb2 = nc.dram_tensor("sb2", [1, D], BF16, kind="ExternalInput").ap()
    esel = nc.dram_tensor("esel", [128, E], F32, kind="ExternalInput").ap()
    yg = nc.dram_tensor("yg", [cap, D], F32, kind="ExternalOutput").ap()
    ys = nc.dram_tensor("ys", [SH, D], F32, kind="ExternalOutput").ap()

    with tile.TileContext(nc) as tc, ExitStack() as ctx:
        const = ctx.enter_context(tc.tile_pool(name="const", bufs=1))
        big = ctx.enter_context(tc.tile_pool(name="big", bufs=1))
        rpool = ctx.enter_context(tc.tile_pool(name="rpool", bufs=2))
        sw1p = ctx.enter_context(tc.tile_pool(name="sw1p", bufs=6))
        sw2p = ctx.enter_context(tc.tile_pool(name="sw2p", bufs=6))
        ost = ctx.enter_context(tc.tile_pool(name="ost", bufs=6))
        psp = ctx.enter_context(tc.tile_pool(name="psp", bufs=7, space="PSUM"))
        lpp = ctx.enter_context(tc.tile_pool(name="lpp", bufs=1, space="PSUM"))

        # ---- small consts (Pool queue: many tiny lines, keep them off the
        # critical SP queue that feeds mm1's first k-tiles) ----
        rw_sb = const.tile([128, KD, E], F32, tag="rw")
        nc.gpsimd.dma_start(out=rw_sb, in_=rw.rearrange("p (k e) -> p k e", k=KD))
        esel_sb = const.tile([128, E], F32, tag="esel")
        nc.gpsimd.dma_start(out=esel_sb, in_=esel)
        if has_rb:
            ones_f = const.tile([1, 128], F32, tag="ones_f")
            nc.vector.memset(ones_f, 1.0)
            rb_sb = const.tile([1, E], F32, tag="rb")
            nc.sync.dma_start(out=rb_sb, in_=rb)
        if has_b1:
            ones_tf = const.tile([1, 512], F32, tag="ones_tf")
            nc.vector.memset(ones_tf, 1.0)
            ones_tok = const.tile([1, 512], BF16, tag="ones_tok")
            nc.vector.tensor_copy(ones_tok, ones_tf[:])
            b1row = const.tile([1, H], BF16, tag="b1row")
            nc.sync.dma_start(out=b1row, in_=b1)
            sb1row = const.tile([1, H], BF16, tag="sb1row")
            nc.sync.dma_start(out=sb1row, in_=sb1)
        if has_b2:
            ones_mf = const.tile([1, 128], F32, tag="ones_mf")
            nc.vector.memset(ones_mf, 1.0)
            onesm_b = const.tile([1, 128], BF16, tag="onesm_b")
            nc.vector.tensor_copy(onesm_b, ones_mf[:])
            b2row = const.tile([1, D], BF16, tag="b2row")
            nc.sync.dma_start(out=b2row, in_=b2)
            sb2row = const.tile([1, D], BF16, tag="sb2row")
            nc.sync.dma_start(out=sb2row, in_=sb2)

        # ---- resident tensors ----
        # Critical path: w1 k-tiles (SP) + xgb k-tiles (Act) feed mm1 k=0
        # within ~2us.  xf/sw follow on SP, w2/xsb on Act.
        # w1 streams in quarter-column pieces ordered to match mm1's j-pair
        # consumption (quarter 0 of every k-tile first); xgb streams chunk-A
        # columns first.  This lines DMA arrival order up with PE demand so
        # the first q-groups start ~1us in instead of waiting on full tiles.
        w1_sb = big.tile([128, KD, H], BF16, tag="w1res")
        w1_r = w1.rearrange("(k p) h -> p k h", p=128)
        xgb_sb = big.tile([128, KD, cap], BF16, tag="xgb")
        xgb_r = xgt_b.rearrange("(k p) t -> p k t", p=128)
        for k in range(KD):
            nc.scalar.dma_start(out=xgb_sb[:, k, 0:512], in_=xgb_r[:, k, 0:512])
        for jq in range(4):
            cs = slice(jq * 512, (jq + 1) * 512)
            for k in range(KD):
                nc.sync.dma_start(out=w1_sb[:, k, cs], in_=w1_r[:, k, cs])
        for k in range(KD):
            nc.scalar.dma_start(out=xgb_sb[:, k, 512:cap], in_=xgb_r[:, k, 512:cap])
        xf_sb = big.tile([128, KD, cap], F32, tag="xf")
        nc.sync.dma_start(out=xf_sb, in_=xgt_f.rearrange("(k p) t -> p k t", p=128))
        xsb_sb = big.tile([128, KD, SH], BF16, tag="xsb")
        nc.scalar.dma_start(out=xsb_sb, in_=xst_b.rearrange("(k p) t -> p k t", p=128))
        w2_sb = big.tile([128, KH, D], BF16, tag="w2res")
        nc.scalar.dma_start(out=w2_sb, in_=w2.rearrange("(k p) d -> p k d", p=128))
        ht = big.tile([128, KH, cap], BF16, tag="ht")

        # ---- expert mm1: ht[j] = gelu(w1.T-block @ xT), w1 resident.
        # j-pair PSUM groups (2 banks) so ~3.5 groups pipeline in 7 slots. ----
        for t0, nt in _chunks(cap):
            for q in range(8):
                phs = []
                for mh in range(2):
                    j = q * 2 + mh
                    ph = psp.tile([128, nt], F32, tag="ps", name=f"ph{t0}_{q}_{mh}")
                    phs.append(ph)
                    if has_b1:
                        nc.tensor.matmul(
                            ph, b1row[:, j * 128 : (j + 1) * 128],
                            ones_tok[:, :nt], start=True, stop=False)
                for k in range(KD):
                    for mh in range(2):
                        j = q * 2 + mh
                        nc.tensor.matmul(
                            phs[mh],
                            w1_sb[:, k, j * 128 : (j + 1) * 128],
                            xgb_sb[:, k, t0 : t0 + nt],
                            start=(k == 0 and not has_b1),
                            stop=(k == KD - 1))
                for mh in range(2):
                    j = q * 2 + mh
                    nc.scalar.activation(ht[:, j, t0 : t0 + nt], phs[mh][:], AF.Gelu)

        # ---- router: one psum bank holds all MTE logit tiles; the fp32
        # matmuls are interleaved between 512-wide bf16 mm2 matmuls so their
        # 512-cycle fp32 weight loads hide under the long moving phases. ----
        lp_all = lpp.tile([128, MTE, E], F32, tag="lp")
        wv = [const.tile([128, 1], F32, tag=f"wv{mt}", name=f"wv{mt}")
              for mt in range(MTE)]

        def router_steps():
            """Yield after each single fp32 router matmul so the caller can
            sandwich them between 512-wide bf16 matmuls (hides ldweights)."""
            for mt in range(MTE):
                ts = slice(mt * 128, (mt + 1) * 128)
                for k in range(KD):
                    nc.tensor.matmul(
                        lp_all[:, mt, :],
                        xf_sb[:, k, ts],
                        rw_sb[:, k, :],
                        start=(k == 0),
                        stop=(k == KD - 1 and not has_rb),
                        skip_group_check=True,
                    )
                    if k < KD - 1:
                        yield 0
                if has_rb:
                    nc.tensor.matmul(lp_all[:, mt, :], ones_f[:], rb_sb[:],
                                     start=False, stop=True, skip_group_check=True)
                l_sb = rpool.tile([128, E], F32, tag="l", name=f"l{mt}")
                nc.vector.tensor_copy(l_sb, lp_all[:, mt, :])
                m1 = rpool.tile([128, 1], F32, tag="m1", name=f"m1_{mt}")
                nc.vector.reduce_max(m1, l_sb[:], axis=X)
                mask1 = rpool.tile([128, E], F32, tag="mask1", name=f"mask1_{mt}")
                nc.vector.tensor_scalar(mask1, l_sb[:], m1[:], None,
                                        op0=AluOpType.is_equal)
                lm = rpool.tile([128, E], F32, tag="lm", name=f"lm{mt}")
                nc.vector.scalar_tensor_tensor(
                    out=lm, in0=mask1[:], scalar=-1e30, in1=l_sb[:],
                    op0=AluOpType.mult, op1=AluOpType.add)
                m2 = rpool.tile([128, 1], F32, tag="m2", name=f"m2_{mt}")
                nc.vector.reduce_max(m2, lm[:], axis=X)
                mask2 = rpool.tile([128, E], F32, tag="mask2", name=f"mask2_{mt}")
                nc.vector.tensor_scalar(mask2, lm[:], m2[:], None,
                                        op0=AluOpType.is_equal)
                dgap = rpool.tile([128, 1], F32, tag="dgap", name=f"dgap{mt}")
                nc.vector.tensor_tensor(dgap, m1[:], m2[:], op=AluOpType.subtract)
                s1 = rpool.tile([128, 1], F32, tag="s1", name=f"s1_{mt}")
                nc.scalar.activation(s1, dgap[:], AF.Sigmoid)
                s2 = rpool.tile([128, 1], F32, tag="s2", name=f"s2_{mt}")
                nc.scalar.activation(s2, dgap[:], AF.Sigmoid, scale=-1.0)
                c1 = rpool.tile([128, E], F32, tag="c1", name=f"c1_{mt}")
                nc.vector.tensor_scalar(c1, mask1[:], s1[:], None,
                                        op0=AluOpType.mult)
                cm = rpool.tile([128, E], F32, tag="cm", name=f"cm{mt}")
                nc.vector.scalar_tensor_tensor(
                    out=cm, in0=mask2[:], scalar=s2[:], in1=c1[:],
                    op0=AluOpType.mult, op1=AluOpType.add)
                wsel = rpool.tile([128, E], F32, tag="wsel", name=f"wsel{mt}")
                nc.vector.tensor_tensor(wsel, cm[:], esel_sb[:], op=AluOpType.mult)
                nc.vector.reduce_sum(wv[mt], wsel[:], axis=X)
                yield 0

        router_it = router_steps()

        def pump_router():
            try:
                next(router_it)
            except StopIteration:
                pass

        yg_r = yg.rearrange("(m p) d -> p m d", p=128)
        ys_r = ys.rearrange("(m p) d -> p m d", p=128)

        def mm2_group(grp, ht_cols, w2src, seed, out_r, scale, gname,
                      interleave_router=False):
            """One mm2 PSUM group: grp m-tiles x 2 n-halves, full k loop."""
            pos = {}
            for mt in grp:
                for n in range(2):
                    po = psp.tile([128, 512], F32, tag="ps", name=f"po{gname}_{mt}_{n}")
                    pos[(mt, n)] = po
                    if seed is not None:
                        nc.tensor.matmul(
                            po, onesm_b[:], seed[:, n * 512 : (n + 1) * 512],
                            start=True, stop=False)
            for k in range(KH):
                w2t = w2src(k)
                for mt in grp:
                    for n in range(2):
                        nc.tensor.matmul(
                            pos[(mt, n)],
                            ht[:, k, ht_cols(mt)],
                            w2t[:, n * 512 : (n + 1) * 512],
                            start=(k == 0 and seed is None),
                            stop=(k == KH - 1))
                    if interleave_router:
                        pump_router()
            for mt in grp:
                for n in range(2):
                    og = ost.tile([128, 512], F32, tag="og",
                                  name=f"og{gname}_{mt}_{n}")
                    # split the two n-half evictions across DVE and ACT so
                    # each group's drain runs on two engines in parallel
                    if scale:
                        if n == 0:
                            nc.vector.tensor_scalar(
                                og, pos[(mt, n)][:], wv[mt][:], None,
                                op0=AluOpType.mult)
                        else:
                            nc.scalar.activation(
                                og, pos[(mt, n)][:], AF.Copy, scale=wv[mt][:])
                    else:
                        if n == 0:
                            nc.vector.tensor_copy(og, pos[(mt, n)][:])
                        else:
                            nc.scalar.activation(og, pos[(mt, n)][:], AF.Copy)
                    nc.scalar.dma_start(
                        out=out_r[:, mt, n * 512 : (n + 1) * 512], in_=og)

        # ---- expert mm2 (m-tile pairs; 128-token tail deferred to the end) ----
        eb2seed = b2row if has_b2 else None
        sb2seed = sb2row if has_b2 else None
        e_w2 = lambda k: w2_sb[:, k, :]
        e_cols = lambda mt: slice(mt * 128, (mt + 1) * 128)
        for g in range(8):
            mm2_group([g], e_cols, e_w2, eb2seed, yg_r, True, f"e{g}",
                      interleave_router=True)

        # ---- shared mm1 (sw1 streamed on the Pool queue, 4-bank j-quads) ----
        for q in range(4):
            phs = []
            for mh in range(4):
                j = q * 4 + mh
                ph = psp.tile([128, SH], F32, tag="ps", name=f"sph{q}_{mh}")
                phs.append(ph)
                if has_b1:
                    nc.tensor.matmul(
                        ph, sb1row[:, j * 128 : (j + 1) * 128],
                        ones_tok[:, :SH], start=True, stop=False)
            for k in range(KD):
                swt = sw1p.tile([128, 512], BF16, tag="sw1t", name=f"sw1_{q}_{k}")
                nc.gpsimd.dma_start(
                    out=swt,
                    in_=sw1[k * 128 : (k + 1) * 128, q * 512 : (q + 1) * 512])
                for mh in range(4):
                    nc.tensor.matmul(
                        phs[mh],
                        swt[:, mh * 128 : (mh + 1) * 128],
                        xsb_sb[:, k, :],
                        start=(k == 0 and not has_b1),
                        stop=(k == KD - 1))
            for mh in range(4):
                j = q * 4 + mh
                nc.scalar.activation(ht[:, j, 0:SH], phs[mh][:], AF.Gelu)

        # ---- shared mm2 (sw2 streamed per pair-group; singles would re-
        # stream sw2 4x and go DMA-bound) ----
        s_cols = lambda mt: slice(mt * 128, (mt + 1) * 128)
        for gi, grp in enumerate([[0, 1], [2, 3]]):
            def s_w2(k, gi=gi):
                swt = sw2p.tile([128, 1024], BF16, tag="sw2t", name=f"sw2_{gi}_{k}")
                nc.gpsimd.dma_start(out=swt, in_=sw2[k * 128 : (k + 1) * 128, :])
                return swt
            mm2_group(grp, s_cols, s_w2, sb2seed, ys_r, False, f"s{gi}")

        # ---- expert mm2 tail (mt=8..): tiny group last -> short drain ----
        for g in range(8, MTE):
            mm2_group([g], e_cols, e_w2, eb2seed, yg_r, True, f"t{g}")

    nc.compile()
    return nc


_programs: dict = {}
LAST_RESULTS = None


def _get_program(key):
    if key not in _programs:
        _programs[key] = build_program(*key)
    return _programs[key]


def kernel(x, router_w, router_b, sw1, sb1, sw2, sb2, ew1, eb1, ew2, eb2):
    x = np.asarray(x, dtype=np.float32)
    flat = np.ascontiguousarray(x.reshape(T, D))
    rw_in = np.ascontiguousarray(np.asarray(router_w, np.float32))
    rb_in = np.asarray(router_b, np.float32).reshape(1, E)

    # Integer dispatch decisions only: which two experts each token visits.
    logits = flat @ rw_in + rb_in
    top1 = np.argmax(logits, axis=1)
    l2m = np.array(logits)
    l2m[np.arange(T), top1] = -np.inf
    top2 = np.argmax(l2m, axis=1)
    idxs = [np.flatnonzero((top1 == e) | (top2 == e)) for e in range(E)]
    maxn = max(len(ix) for ix in idxs)
    cap = max(DEFAULT_CAP, ((maxn + 127) // 128) * 128)

    has_b1 = bool(np.any(sb1)) or bool(np.any(eb1))
    has_b2 = bool(np.any(sb2)) or bool(np.any(eb2))
    has_rb = bool(np.any(router_b))
    nc = _get_program((cap, has_b1, has_b2, has_rb))

    # rw pre-permuted to [128, KD*E] (partition-major) for a wide-line DMA
    rw_perm = np.ascontiguousarray(
        rw_in.reshape(KD, 128, E).transpose(1, 0, 2).reshape(128, KD * E))

    sw1b = np.ascontiguousarray(np.asarray(sw1)).astype(BF)
    sw2b = np.ascontiguousarray(np.asarray(sw2)).astype(BF)
    sb1b = np.asarray(sb1).reshape(1, H).astype(BF)
    sb2b = np.asarray(sb2).reshape(1, D).astype(BF)
    ew1a = np.asarray(ew1)
    ew2a = np.asarray(ew2)
    eb1a = np.asarray(eb1)
    eb2a = np.asarray(eb2)

    in_maps = []
    for c in range(NCORES):
        ix = idxs[c]
        xg = np.zeros((cap, D), np.float32)
        xg[: len(ix)] = flat[ix]
        xgt = np.ascontiguousarray(xg.T)
        esel_c = np.zeros((128, E), np.float32)
        esel_c[:, c] = 1.0
        in_maps.append({
            "xgt_f": xgt,
            "xgt_b": xgt.astype(BF),
            "xst_b": np.ascontiguousarray(flat[c * SH : (c + 1) * SH].T).astype(BF),
            "rw": rw_perm,
            "rb": rb_in,
            "w1": np.ascontiguousarray(ew1a[c]).astype(BF),
            "w2": np.ascontiguousarray(ew2a[c]).astype(BF),
            "sw1": sw1b,
            "sw2": sw2b,
            "b1": np.asarray(eb1a[c]).reshape(1, H).astype(BF),
            "b2": np.asarray(eb2a[c]).reshape(1, D).astype(BF),
            "sb1": sb1b,
            "sb2": sb2b,
            "esel": esel_c,
        })

    res = None
    for attempt in range(5):
        try:
            res = run_bass_kernel_spmd(nc, in_maps, core_ids=list(range(NCORES)))
            break
        except Exception:
            if attempt == 4:
                raise
            import time as _time
            _time.sleep(25)  # wedged-device windows recover after ~1-2 min
    global LAST_RESULTS
    LAST_RESULTS = res

    y = np.empty((T, D), np.float32)
    for c in range(NCORES):
        y[c * SH : (c + 1) * SH] = res.results[c]["ys"]
    for c in range(NCORES):
        ix = idxs[c]
        y[ix] += res.results[c]["yg"][: len(ix)]
    return y.reshape(B, S, D)
